# revision 39
# baseline (speedup 1.0000x reference)
"""AlphaMixerAttentionHeads TRN2 kernel, v2.

Algebraic structure (inherited from the validated baseline):
 - alpha is i-independent, so it collapses to a per-(b,h) vector u over o;
   the output is constant across sequence positions and equals
   m_3 = sum_o H3[:,o] u_3[o].
 - W rows are L1-normalized, so all per-token scales cancel through the
   NNMF recurrence, which runs on raw clipped xe.

v2 changes vs the baseline:
 - All W preparation (row-normalize, transpose, iter-1 fold, M2 = W2Tp@W2)
   happens on the HOST in numpy; the device receives ready bf16 lhsT tiles.
   This removes the on-device transpose/normalize chain that sat on the
   critical path between the wpk DMA and the first NNMF matmul.
 - rec2 = M2^T xe is computed straight from xe (M2 folded on host), so
   iteration 2 does not wait for the H1 copy.
 - DMA queues: x + ewc2 on the sync queue, ewT + const packs on the scalar
   queue (smallest/earliest-needed first), out-projection weights on the
   gpsimd queue triggered mid-kernel -- the embed matmuls no longer wait on
   the (large, late-needed) out-projection DMAs.
 - q = xe / rec and hri = hri_a / B are single DVE divide ops (no separate
   reciprocal+multiply).
 - alpha rounds: per-chunk accumulator matmuls (W^T m accumulated in PSUM
   across chunks), vblk built on DVE, g for chunks 1-2 copied to bf16 so the
   scalar_tensor_tensor accumulation runs in the DVE 2-byte fast mode; the
   t tiles are bf16.

Sharding: 8 cores; core c handles batch c//4 and heads 3*(c%4)..3*(c%4)+2.
Host sums 4 partial output projections per batch, adds out_b, broadcasts
over the sequence axis.

On-core layout: [feature, token], one [128, 1536] tile set: cols 0..1023 =
heads A,B (partitions 0-63 = A, 64-127 = B); cols 1024..1535 = head C
split-token (partitions 0-63 = tokens 0-511, 64-127 = tokens 512-1023).
"""

import sys

sys.path.insert(0, "/opt/trn_rl_repo")

import numpy as np

B, S, FIN, E, H = 2, 1024, 768, 768, 12
DH = 64
HPC = 3          # heads per core
EPC = HPC * DH   # embed channels per core (192)
NCORES = 8
MIN_POS = 1e-6
EPS = 1e-20
NT = 1536        # merged token columns: 1024 pair + 512 C-split
KT = FIN // 128  # 6 contraction tiles for the embed matmul
CH = 512         # pipeline chunk columns
# wb packed bf16 columns: W2Tpb | M2b | W2b | W2Tb | Wstk2b | idstkb | ones2b
WB_COLS = 128 + 128 + 128 + 128 + 128 + 64 + 128

_CACHE = {}


def _build_nc():
    import concourse.bacc as bacc
    import concourse.mybir as mybir
    from concourse.tile import TileContext

    f32 = mybir.dt.float32
    f32r = mybir.dt.float32r
    bf16 = mybir.dt.bfloat16
    Alu = mybir.AluOpType
    Act = mybir.ActivationFunctionType

    nc = bacc.Bacc()

    fp8 = mybir.dt.float8e4
    d_xT = nc.declare_dram_parameter("xT", [128, KT, S], fp8, isOutput=False)
    d_ewT = nc.declare_dram_parameter("ewT", [128, KT, EPC], fp8, isOutput=False)
    d_ewc2 = nc.declare_dram_parameter("ewc2", [128, KT, 128], fp8, isOutput=False)
    d_wb = nc.declare_dram_parameter("wb", [128, WB_COLS], bf16, isOutput=False)
    d_wf = nc.declare_dram_parameter("wf", [128, 2], f32, isOutput=False)
    d_owa = nc.declare_dram_parameter("owa", [128, FIN], bf16, isOutput=False)
    d_owc = nc.declare_dram_parameter("owc", [64, FIN], bf16, isOutput=False)
    d_y = nc.declare_dram_parameter("y", [1, FIN], f32, isOutput=True)

    CHUNKS = ((0, 512), (512, 1024), (1024, 1536))

    with TileContext(nc) as tc:
        with (
            tc.tile_pool(name="const", bufs=1) as const,
            tc.tile_pool(name="xch", bufs=3) as xch,
            tc.tile_pool(name="work", bufs=1) as work,
            tc.tile_pool(name="hbuf", bufs=2) as hbuf,
            tc.tile_pool(name="tbuf", bufs=2) as tbuf,
            tc.tile_pool(name="pmm", bufs=4, space="PSUM") as pmm,
            tc.tile_pool(name="pt", bufs=1, space="PSUM") as pt,
        ):
            # ---- DMA triggers. sync queue: x tiles; scalar queue: embed
            # weights + const packs. owa/owc go on the gpsimd queue later.
            xts = []
            for i in range(3):
                xt = xch.tile([128, 2, S], fp8, tag="xch")
                nc.sync.dma_start(out=xt, in_=d_xT[:, 2 * i:2 * i + 2, :])
                xts.append(xt)
            ewT_sb = const.tile([128, KT, EPC], fp8)
            nc.scalar.dma_start(out=ewT_sb, in_=d_ewT[:, :, :])
            ewc2_sb = const.tile([128, KT, 128], fp8)
            nc.scalar.dma_start(out=ewc2_sb, in_=d_ewc2[:, :, :])
            wb = const.tile([128, WB_COLS], bf16)
            nc.scalar.dma_start(out=wb, in_=d_wb[:, :])
            wf = const.tile([128, 2], f32)
            nc.scalar.dma_start(out=wf, in_=d_wf[:, :])

            W2Tpb = wb[:, 0:128]
            M2b = wb[:, 128:256]
            W2b = wb[:, 256:384]
            W2Tb = wb[:, 384:512]
            Wstk2b = wb[:, 512:640]
            idstkb = wb[:, 640:704]
            ones2b = wb[:, 704:832]
            ebm_p = wf[:, 0:1]
            ebm_c2 = wf[:, 1:2]

            # ---- embed matmuls: ep = pair heads [128, 1024];
            # psC = head C split-token [128, 512].
            ep = pmm.tile([128, 1024], f32, tag="ep", bufs=1)
            psC = pmm.tile([128, CH], f32, tag="pc", bufs=1)
            DR = mybir.MatmulPerfMode.DoubleRow
            for g in range(3):
                xt = xts[g]
                lhsP2 = ewT_sb[:, 2 * g:2 * g + 2, 0:128]
                st = dict(start=(g == 0), stop=(g == 2), perf_mode=DR)
                nc.tensor.matmul(
                    out=ep[:, 0:512], lhsT=lhsP2, rhs=xt[:, :, 0:512], **st
                )
                nc.tensor.matmul(
                    out=ep[:, 512:1024], lhsT=lhsP2, rhs=xt[:, :, 512:1024], **st
                )
                nc.tensor.matmul(
                    out=psC, lhsT=ewc2_sb[:, 2 * g:2 * g + 2, :],
                    rhs=xt[:, :, 512:1024], skip_group_check=True, **st,
                )
                nc.tensor.matmul(
                    out=psC[0:64, :], lhsT=ewT_sb[:, 2 * g:2 * g + 2, 128:192],
                    rhs=xt[:, :, 0:512], start=False, stop=(g == 2),
                    perf_mode=DR, skip_group_check=True,
                )

            # ---- xe = relu(embed + eb - MIN_POS) on ACT (bias pre-folded)
            xe = work.tile([128, NT], bf16)
            nc.scalar.activation(
                out=xe[:, 0:512], in_=ep[:, 0:512], func=Act.Relu,
                bias=ebm_p, scale=1.0 / 64.0,
            )
            nc.scalar.activation(
                out=xe[:, 512:1024], in_=ep[:, 512:1024], func=Act.Relu,
                bias=ebm_p, scale=1.0 / 64.0,
            )
            nc.scalar.activation(
                out=xe[:, 1024:1536], in_=psC, func=Act.Relu,
                bias=ebm_c2, scale=1.0 / 64.0,
            )

            # ---- NNMF iter 1: H1 = xe @ (W^T * rec1r); row sums sx of xe
            z1s = []
            for lo, hi in CHUNKS:
                z = pmm.tile([128, CH], f32, tag="mm")
                nc.tensor.matmul(out=z, lhsT=W2Tpb, rhs=xe[:, lo:hi])
                z1s.append(z)
            # rec2 straight from xe via the host-folded M2
            rec2s = []
            for lo, hi in CHUNKS:
                r = pmm.tile([128, CH], f32, tag="mm")
                nc.tensor.matmul(out=r, lhsT=M2b, rhs=xe[:, lo:hi])
                rec2s.append(r)
            sxs_ps = []
            for lo, hi in CHUNKS:
                sx = pmm.tile([128, CH], f32, tag="mm")
                nc.tensor.matmul(out=sx, lhsT=ones2b, rhs=xe[:, lo:hi])
                sxs_ps.append(sx)
            H1 = hbuf.tile([128, NT], bf16, tag="h")
            for ci, (lo, hi) in enumerate(CHUNKS):
                nc.scalar.activation(out=H1[:, lo:hi], in_=z1s[ci], func=Act.Copy)
            sxs = work.tile([128, NT], f32)
            for ci, (lo, hi) in enumerate(CHUNKS):
                nc.scalar.activation(out=sxs[:, lo:hi], in_=sxs_ps[ci], func=Act.Copy)

            # ---- NNMF iter 2: q2 = xe / rec2; H2 = H1 * (W2T^T q2); s2
            rr2 = work.tile([128, NT], f32, tag="rr2")
            for ci, (lo, hi) in enumerate(CHUNKS):
                nc.vector.reciprocal_approx_fast(out=rr2[:, lo:hi], in_=rec2s[ci])
            # chunk 2 is the tail of every chain: its q runs on DVE (GP queue
            # would delay it) and its H update reads PSUM directly (skips the
            # ACT-copy hop); chunks 0/1 keep the throughput-optimal routing.
            q2 = work.tile([128, NT], bf16, tag="q2")
            for ci, (lo, hi) in enumerate(CHUNKS):
                eng = nc.vector if ci == 2 else nc.gpsimd
                eng.tensor_tensor(
                    out=q2[:, lo:hi], in0=xe[:, lo:hi], in1=rr2[:, lo:hi],
                    op=Alu.mult,
                )
            z2s = []
            z2b = work.tile([128, NT], bf16, tag="z2b")
            for lo, hi in CHUNKS:
                z = pmm.tile([128, CH], f32, tag="mm")
                nc.tensor.matmul(out=z, lhsT=W2Tb, rhs=q2[:, lo:hi])
                z2s.append(z)
            for ci, (lo, hi) in enumerate(CHUNKS[:2]):
                nc.scalar.activation(out=z2b[:, lo:hi], in_=z2s[ci], func=Act.Copy)
            H2 = hbuf.tile([128, NT], bf16, tag="h")
            nc.vector.tensor_tensor(
                out=H2[:, 1024:1536], in0=H1[:, 1024:1536], in1=z2s[2],
                op=Alu.mult,
            )
            for ci, (lo, hi) in enumerate(CHUNKS[:2]):
                nc.vector.tensor_tensor(
                    out=H2[:, lo:hi], in0=H1[:, lo:hi], in1=z2b[:, lo:hi],
                    op=Alu.mult,
                )
            s2_ps = []
            for lo, hi in CHUNKS:
                s2 = pmm.tile([128, CH], f32, tag="mm")
                nc.tensor.matmul(out=s2, lhsT=ones2b, rhs=H2[:, lo:hi])
                s2_ps.append(s2)
            # Bden = sx * s2 (hri denominator); rhoB = 1/Bden
            Bden = work.tile([128, NT], f32)
            for ci, (lo, hi) in enumerate(CHUNKS):
                nc.vector.tensor_tensor(
                    out=Bden[:, lo:hi], in0=sxs[:, lo:hi], in1=s2_ps[ci],
                    op=Alu.mult,
                )
            rhoB = work.tile([128, NT], f32)
            for lo, hi in CHUNKS:
                nc.vector.reciprocal_approx_fast(
                    out=rhoB[:, lo:hi], in_=Bden[:, lo:hi]
                )

            # ---- NNMF iter 3: rec3, q3, H3, s3; hri = (rec3*xe)/B
            rec3s_ps = [None, None, None]
            for ci in (2, 0, 1):
                lo, hi = CHUNKS[ci]
                r = pmm.tile([128, CH], f32, tag="mm")
                nc.tensor.matmul(out=r, lhsT=W2b, rhs=H2[:, lo:hi])
                rec3s_ps[ci] = r
            q3 = work.tile([128, NT], bf16, tag="q3")
            rr3 = work.tile([128, NT], f32, tag="rr3")
            rec3b = work.tile([128, NT], bf16)
            for ci in (2, 0, 1):
                lo, hi = CHUNKS[ci]
                nc.vector.reciprocal_approx_fast(out=rr3[:, lo:hi], in_=rec3s_ps[ci])
                nc.scalar.activation(
                    out=rec3b[:, lo:hi], in_=rec3s_ps[ci], func=Act.Copy
                )
            for ci, (lo, hi) in enumerate(CHUNKS):
                eng = nc.vector if ci == 2 else nc.gpsimd
                eng.tensor_tensor(
                    out=q3[:, lo:hi], in0=xe[:, lo:hi], in1=rr3[:, lo:hi],
                    op=Alu.mult,
                )
            z3s = []
            z3b = work.tile([128, NT], bf16, tag="z3b")
            for lo, hi in CHUNKS:
                z = pmm.tile([128, CH], f32, tag="mm")
                nc.tensor.matmul(out=z, lhsT=W2Tb, rhs=q3[:, lo:hi])
                z3s.append(z)
            for ci, (lo, hi) in enumerate(CHUNKS[:2]):
                nc.scalar.activation(out=z3b[:, lo:hi], in_=z3s[ci], func=Act.Copy)
            H3 = hbuf.tile([128, NT], bf16, tag="h")
            nc.vector.tensor_tensor(
                out=H3[:, 1024:1536], in0=H2[:, 1024:1536], in1=z3s[2],
                op=Alu.mult,
            )
            for ci, (lo, hi) in enumerate(CHUNKS[:2]):
                nc.vector.tensor_tensor(
                    out=H3[:, lo:hi], in0=H2[:, lo:hi], in1=z3b[:, lo:hi],
                    op=Alu.mult,
                )
            # hri_a = rec3 * xe on DVE (all-bf16: probes the 2x fast mode).
            # owa/owc DMAs are anchored on H2 via tiny writes into the
            # destination tiles so the scheduler cannot hoist them into the
            # input-DMA window (they'd steal bandwidth from xT/ewT).
            hria = work.tile([128, NT], bf16)
            owa = const.tile([128, FIN], bf16)
            owc = const.tile([64, FIN], bf16)
            nc.gpsimd.tensor_copy(out=owa[:, 0:1], in_=H2[:, 0:1])
            nc.gpsimd.tensor_copy(out=owc[:, 0:1], in_=H2[0:64, 0:1])
            nc.gpsimd.dma_start(out=owa, in_=d_owa[:, :])
            nc.gpsimd.dma_start(out=owc, in_=d_owc[:, :])
            # hria = rec3 * xe (all-bf16 2x TT); hri = hria * rhoB
            for lo, hi in CHUNKS:
                nc.vector.tensor_tensor(
                    out=hria[:, lo:hi], in0=rec3b[:, lo:hi], in1=xe[:, lo:hi],
                    op=Alu.mult,
                )
            hri = work.tile([128, NT], bf16)
            for lo, hi in CHUNKS:
                nc.gpsimd.tensor_tensor(
                    out=hri[:, lo:hi], in0=hria[:, lo:hi], in1=rhoB[:, lo:hi],
                    op=Alu.mult,
                )
            s3_ps = [None, None, None]
            for ci in (2, 0, 1):
                lo, hi = CHUNKS[ci]
                s3 = pmm.tile([128, CH], f32, tag="mm")
                nc.tensor.matmul(out=s3, lhsT=ones2b, rhs=H3[:, lo:hi])
                s3_ps[ci] = s3
            u0 = work.tile([128, NT], f32)
            for ci in (2, 0, 1):
                lo, hi = CHUNKS[ci]
                nc.vector.reciprocal_approx_fast(out=u0[:, lo:hi], in_=s3_ps[ci])

            # ---- alpha fixed point: 4 accumulation passes, 3 v/g rounds
            vv = pt.tile([128, 4], f32, tag="v", bufs=1)
            c_p = work.tile([128, 1], f32)
            c_cc = work.tile([128, 1], f32)
            t_prev = None
            g_ps = None
            for it in range(4):
                lastit = it == 3
                t = tbuf.tile([128, NT], bf16, tag="t")
                in0 = H3 if it == 0 else t_prev
                macc = []
                mbs = []
                vps = vv[:, 0:1]
                vcs = vv[:, 1:2]
                for ci, (lo, hi) in enumerate(CHUNKS):
                    in1 = u0[:, lo:hi] if it == 0 else g_ps[ci]
                    m = work.tile([128, 1], f32, tag=f"m{it}{ci}")
                    nc.vector.scalar_tensor_tensor(
                        out=t[:, lo:hi], in0=in0[:, lo:hi], scalar=1.0,
                        in1=in1, op0=Alu.mult, op1=Alu.mult, accum_out=m,
                    )
                    macc.append(m)
                    if lastit:
                        continue
                    # cast (on ACT, off the DVE serial path) + accumulated v
                    # matmul immediately per chunk so the v chain overlaps
                    # the remaining STTs
                    mb = work.tile([128, 1], bf16, tag=f"mb{it}{ci}")
                    nc.scalar.activation(out=mb, in_=m, func=Act.Copy)
                    mbs.append(mb)
                    if ci < 2:
                        nc.tensor.matmul(
                            out=vps, lhsT=W2b, rhs=mb,
                            start=(ci == 0), stop=(ci == 1),
                            skip_group_check=True,
                        )
                    else:
                        nc.tensor.matmul(
                            out=vcs, lhsT=Wstk2b, rhs=mb, skip_group_check=True
                        )
                t_prev = t
                if lastit:
                    nc.vector.tensor_tensor(
                        out=c_p, in0=macc[0], in1=macc[1], op=Alu.add
                    )
                    nc.vector.tensor_copy(out=c_cc, in_=macc[2])
                    break
                v_p = work.tile([128, 1], f32, tag="v_p")
                v_c = work.tile([128, 1], f32, tag="v_c")
                nc.vector.reciprocal_approx_fast(out=v_p, in_=vps)
                nc.vector.reciprocal_approx_fast(out=v_c, in_=vcs)
                vblk = work.tile([128, 128], bf16, tag="vblk")
                vblkC = work.tile([128, 128], bf16, tag="vblkC")
                nc.vector.tensor_scalar(
                    out=vblk, in0=ones2b, scalar1=v_p, scalar2=None, op0=Alu.mult
                )
                nc.scalar.activation(
                    out=vblkC, in_=ones2b, func=Act.Copy, scale=v_c
                )
                g_ps = []
                for ci, (lo, hi) in enumerate(CHUNKS):
                    g = pmm.tile([128, CH], f32, tag="mm")
                    nc.tensor.matmul(
                        out=g, lhsT=(vblkC if ci == 2 else vblk),
                        rhs=hri[:, lo:hi],
                    )
                    g_ps.append(g)

            # fold the C accumulator's split halves: c_c[f] = acc[f]+acc[64+f]
            c_ccb = work.tile([128, 1], bf16)
            nc.vector.tensor_copy(out=c_ccb, in_=c_cc)
            fc = vv[0:64, 2:3]
            nc.tensor.matmul(out=fc, lhsT=idstkb, rhs=c_ccb, skip_group_check=True)
            c_c = work.tile([64, 1], bf16)
            nc.scalar.activation(out=c_c, in_=fc, func=Act.Copy)

            # ---- output projection partial: y_row = c^T @ owT  [1, FIN]
            c_pr = work.tile([128, 1], bf16)
            nc.vector.tensor_copy(out=c_pr, in_=c_p)
            py0 = pmm.tile([1, 512], f32, tag="mm")
            py1 = pmm.tile([1, 256], f32, tag="mm")
            for py, (lo, hi) in ((py0, (0, 512)), (py1, (512, FIN))):
                nc.tensor.matmul(
                    out=py, lhsT=c_pr, rhs=owa[:, lo:hi], start=True, stop=False
                )
                nc.tensor.matmul(
                    out=py, lhsT=c_c, rhs=owc[:, lo:hi], start=False, stop=True
                )
            y_sb = work.tile([1, FIN], f32)
            nc.scalar.activation(out=y_sb[:, 0:512], in_=py0, func=Act.Copy)
            nc.scalar.activation(out=y_sb[:, 512:FIN], in_=py1, func=Act.Copy)
            nc.sync.dma_start(out=d_y[:, :], in_=y_sb)

    nc.finalize()
    return nc


def _make_in_maps(x, embed_w, embed_b, nnmf_w, out_w):
    import ml_dtypes

    def to_fp8(a):
        return np.ascontiguousarray(a).astype(ml_dtypes.float8_e4m3fn)

    def to_bf16(a):
        return np.ascontiguousarray(a).astype(ml_dtypes.bfloat16)

    # host W-prep (shared across heads/cores)
    Wn = nnmf_w / np.maximum(nnmf_w.sum(1, keepdims=True), EPS)  # [f,d]
    rec1r = 64.0 / np.maximum(Wn.sum(0), EPS)                    # [d]
    W2Tp = Wn.T * rec1r[:, None]                                 # [d,f]
    M2 = W2Tp @ Wn                                               # [d,d']
    W2T = Wn.T                                                   # [d,f]

    def blkdiag(A):
        Z = np.zeros((128, 128), np.float32)
        Z[0:64, 0:64] = A
        Z[64:128, 64:128] = A
        return Z

    idstk = np.zeros((128, 64), np.float32)
    for k in range(128):
        idstk[k, k % 64] = 1.0
    ones2 = blkdiag(np.ones((64, 64), np.float32))
    Wstk2 = np.zeros((128, 128), np.float32)
    Wstk2[0:64, 0:64] = Wn
    Wstk2[64:128, 0:64] = Wn
    Wstk2[0:64, 64:128] = Wn
    Wstk2[64:128, 64:128] = Wn

    wbpack = np.concatenate(
        [blkdiag(W2Tp), blkdiag(M2), blkdiag(Wn), blkdiag(W2T),
         Wstk2, idstk, ones2],
        axis=1,
    ).astype(np.float32)
    wb = to_bf16(wbpack)

    in_maps = []
    for c in range(NCORES):
        b = c // 4
        hg = c % 4
        esl = slice(EPC * hg, EPC * (hg + 1))
        # xT packed [128, KT, S]: (p, k, t) = x[b, t, 128k+p]
        xT = np.ascontiguousarray(
            x[b].T.reshape(KT, 128, S).transpose(1, 0, 2)
        )
        ewT = np.ascontiguousarray(
            embed_w[esl, :].T.reshape(KT, 128, EPC).transpose(1, 0, 2)
        )
        ewc2 = np.zeros((128, KT, 128), np.float32)
        ewc2[:, :, 64:128] = ewT[:, :, 128:192]
        ebm = embed_b[esl] - MIN_POS
        wf = np.zeros((128, 2), np.float32)
        wf[:, 0] = ebm[0:128]
        wf[0:64, 1] = ebm[128:192]
        wf[64:128, 1] = ebm[128:192]
        owT = out_w[:, esl].T  # [192, FIN]
        in_maps.append({
            "xT": to_fp8(xT),
            "ewT": to_fp8(ewT * 64.0),
            "ewc2": to_fp8(ewc2 * 64.0),
            "wb": wb,
            "wf": wf,
            "owa": to_bf16(owT[0:128, :]),
            "owc": to_bf16(owT[128:192, :]),
        })
    return in_maps


def _ensure_ntff_hook():
    """The agent image's antenv lacks axon_hooks; synthesize it so
    run_bass_kernel_spmd(trace=True) can reach the ctypes NTFF hook."""
    import sys as _sys
    import types

    if "antenv.axon_hooks" in _sys.modules:
        return
    mod = types.ModuleType("antenv.axon_hooks")
    holder = [None]
    mod.set_axon_ntff_profile_hook = lambda h: holder.__setitem__(0, h)
    mod.get_axon_ntff_profile_hook = lambda: holder[0]
    _sys.modules["antenv.axon_hooks"] = mod
    try:
        import antenv

        antenv.axon_hooks = mod
    except ImportError:
        pass
    from trn_agent_boot.trn_boot import _ntff_profile_via_ctypes

    mod.set_axon_ntff_profile_hook(
        _ntff_profile_via_ctypes("/opt/axon/libaxon_pjrt.so")
    )


def _run(inputs, trace=False):
    from concourse import bass_utils

    if trace:
        _ensure_ntff_hook()
    if "nc" not in _CACHE:
        _CACHE["nc"] = _build_nc()
    nc = _CACHE["nc"]
    in_maps = _make_in_maps(
        inputs["x"].astype(np.float32),
        inputs["embed_w"].astype(np.float32),
        inputs["embed_b"].astype(np.float32),
        inputs["nnmf_w"].astype(np.float32),
        inputs["out_w"].astype(np.float32),
    )
    res = bass_utils.run_bass_kernel_spmd(
        nc, in_maps, core_ids=list(range(NCORES)), trace=trace
    )
    out_b = inputs["out_b"].astype(np.float32)
    y = np.zeros((B, S, FIN), np.float32)
    for bi in range(B):
        acc = np.zeros((FIN,), np.float64)
        for c in range(4 * bi, 4 * bi + 4):
            arr = np.asarray(res.results[c]["y"])  # [1, FIN]
            acc += arr.reshape(FIN)
        y[bi, :, :] = (acc + out_b).astype(np.float32)[None, :]
    return y, res


def kernel(**inputs):
    y, _ = _run(inputs, trace=False)
    return y


# revision 41
# speedup vs baseline: 1.0060x; 1.0060x over previous
"""AlphaMixerAttentionHeads TRN2 kernel, v2.

Algebraic structure (inherited from the validated baseline):
 - alpha is i-independent, so it collapses to a per-(b,h) vector u over o;
   the output is constant across sequence positions and equals
   m_3 = sum_o H3[:,o] u_3[o].
 - W rows are L1-normalized, so all per-token scales cancel through the
   NNMF recurrence, which runs on raw clipped xe.

v2 changes vs the baseline:
 - All W preparation (row-normalize, transpose, iter-1 fold, M2 = W2Tp@W2)
   happens on the HOST in numpy; the device receives ready bf16 lhsT tiles.
   This removes the on-device transpose/normalize chain that sat on the
   critical path between the wpk DMA and the first NNMF matmul.
 - rec2 = M2^T xe is computed straight from xe (M2 folded on host), so
   iteration 2 does not wait for the H1 copy.
 - DMA queues: x + ewc2 on the sync queue, ewT + const packs on the scalar
   queue (smallest/earliest-needed first), out-projection weights on the
   gpsimd queue triggered mid-kernel -- the embed matmuls no longer wait on
   the (large, late-needed) out-projection DMAs.
 - q = xe / rec and hri = hri_a / B are single DVE divide ops (no separate
   reciprocal+multiply).
 - alpha rounds: per-chunk accumulator matmuls (W^T m accumulated in PSUM
   across chunks), vblk built on DVE, g for chunks 1-2 copied to bf16 so the
   scalar_tensor_tensor accumulation runs in the DVE 2-byte fast mode; the
   t tiles are bf16.

Sharding: 8 cores; core c handles batch c//4 and heads 3*(c%4)..3*(c%4)+2.
Host sums 4 partial output projections per batch, adds out_b, broadcasts
over the sequence axis.

On-core layout: [feature, token], one [128, 1536] tile set: cols 0..1023 =
heads A,B (partitions 0-63 = A, 64-127 = B); cols 1024..1535 = head C
split-token (partitions 0-63 = tokens 0-511, 64-127 = tokens 512-1023).
"""

import sys

sys.path.insert(0, "/opt/trn_rl_repo")

import numpy as np

B, S, FIN, E, H = 2, 1024, 768, 768, 12
DH = 64
HPC = 3          # heads per core
EPC = HPC * DH   # embed channels per core (192)
NCORES = 8
MIN_POS = 1e-6
EPS = 1e-20
NT = 1536        # merged token columns: 1024 pair + 512 C-split
KT = FIN // 128  # 6 contraction tiles for the embed matmul
CH = 512         # pipeline chunk columns
# wb packed bf16 columns: W2Tpb | M2b | W2b | W2Tb | Wstk2b | idstkb | ones2b
WB_COLS = 128 + 128 + 128 + 128 + 128 + 64 + 128

_CACHE = {}


def _build_nc():
    import concourse.bacc as bacc
    import concourse.mybir as mybir
    from concourse.tile import TileContext

    f32 = mybir.dt.float32
    f32r = mybir.dt.float32r
    bf16 = mybir.dt.bfloat16
    Alu = mybir.AluOpType
    Act = mybir.ActivationFunctionType

    nc = bacc.Bacc()

    fp8 = mybir.dt.float8e4
    d_xT = nc.declare_dram_parameter("xT", [128, KT, S], fp8, isOutput=False)
    d_ewT = nc.declare_dram_parameter("ewT", [128, KT, EPC], fp8, isOutput=False)
    d_ewc2 = nc.declare_dram_parameter("ewc2", [128, KT, 128], fp8, isOutput=False)
    d_wb = nc.declare_dram_parameter("wb", [128, WB_COLS], bf16, isOutput=False)
    d_wf = nc.declare_dram_parameter("wf", [128, 2], f32, isOutput=False)
    d_owa = nc.declare_dram_parameter("owa", [128, FIN], bf16, isOutput=False)
    d_owc = nc.declare_dram_parameter("owc", [64, FIN], bf16, isOutput=False)
    d_y = nc.declare_dram_parameter("y", [1, FIN], f32, isOutput=True)

    CHUNKS = ((0, 512), (512, 1024), (1024, 1536))

    with TileContext(nc) as tc:
        with (
            tc.tile_pool(name="const", bufs=1) as const,
            tc.tile_pool(name="xch", bufs=3) as xch,
            tc.tile_pool(name="work", bufs=1) as work,
            tc.tile_pool(name="hbuf", bufs=2) as hbuf,
            tc.tile_pool(name="tbuf", bufs=2) as tbuf,
            tc.tile_pool(name="pmm", bufs=4, space="PSUM") as pmm,
            tc.tile_pool(name="pt", bufs=1, space="PSUM") as pt,
        ):
            # ---- DMA triggers. sync queue: x tiles in HALF-TOKEN pieces so
            # each embed matmul (which consumes one token half) can start as
            # soon as its half landed; scalar queue: embed weights. wb is
            # anchored on the first x piece so its 213KB doesn't compete for
            # HBM bandwidth in the critical head window (needed only by z1).
            xts = []
            for i in range(3):
                xt = xch.tile([128, 2, S], fp8, tag="xch")
                nc.sync.dma_start(
                    out=xt[:, :, 0:512], in_=d_xT[:, 2 * i:2 * i + 2, 0:512]
                )
                nc.sync.dma_start(
                    out=xt[:, :, 512:1024], in_=d_xT[:, 2 * i:2 * i + 2, 512:1024]
                )
                xts.append(xt)
            ewT_sb = const.tile([128, KT, EPC], fp8)
            nc.scalar.dma_start(out=ewT_sb, in_=d_ewT[:, :, :])
            ewc2_sb = const.tile([128, KT, 128], fp8)
            nc.scalar.dma_start(out=ewc2_sb, in_=d_ewc2[:, :, :])
            wf = const.tile([128, 2], f32)
            nc.scalar.dma_start(out=wf, in_=d_wf[:, :])
            wb = const.tile([128, WB_COLS], bf16)
            nc.gpsimd.tensor_copy(out=wb[:, 0:1], in_=xts[0][:, 0, 0:1])
            nc.scalar.dma_start(out=wb, in_=d_wb[:, :])

            W2Tpb = wb[:, 0:128]
            M2b = wb[:, 128:256]
            W2b = wb[:, 256:384]
            W2Tb = wb[:, 384:512]
            Wstk2b = wb[:, 512:640]
            idstkb = wb[:, 640:704]
            ones2b = wb[:, 704:832]
            ebm_p = wf[:, 0:1]
            ebm_c2 = wf[:, 1:2]

            # ---- embed matmuls: ep = pair heads [128, 1024];
            # psC = head C split-token [128, 512].
            ep = pmm.tile([128, 1024], f32, tag="ep", bufs=1)
            psC = pmm.tile([128, CH], f32, tag="pc", bufs=1)
            DR = mybir.MatmulPerfMode.DoubleRow
            for g in range(3):
                xt = xts[g]
                lhsP2 = ewT_sb[:, 2 * g:2 * g + 2, 0:128]
                st = dict(start=(g == 0), stop=(g == 2), perf_mode=DR)
                nc.tensor.matmul(
                    out=ep[:, 0:512], lhsT=lhsP2, rhs=xt[:, :, 0:512], **st
                )
                nc.tensor.matmul(
                    out=ep[:, 512:1024], lhsT=lhsP2, rhs=xt[:, :, 512:1024], **st
                )
                nc.tensor.matmul(
                    out=psC, lhsT=ewc2_sb[:, 2 * g:2 * g + 2, :],
                    rhs=xt[:, :, 512:1024], skip_group_check=True, **st,
                )
                nc.tensor.matmul(
                    out=psC[0:64, :], lhsT=ewT_sb[:, 2 * g:2 * g + 2, 128:192],
                    rhs=xt[:, :, 0:512], start=False, stop=(g == 2),
                    perf_mode=DR, skip_group_check=True,
                )

            # ---- xe = relu(embed + eb - MIN_POS) on ACT (bias pre-folded)
            xe = work.tile([128, NT], bf16)
            nc.scalar.activation(
                out=xe[:, 0:512], in_=ep[:, 0:512], func=Act.Relu,
                bias=ebm_p, scale=1.0 / 64.0,
            )
            nc.scalar.activation(
                out=xe[:, 512:1024], in_=ep[:, 512:1024], func=Act.Relu,
                bias=ebm_p, scale=1.0 / 64.0,
            )
            nc.scalar.activation(
                out=xe[:, 1024:1536], in_=psC, func=Act.Relu,
                bias=ebm_c2, scale=1.0 / 64.0,
            )

            # ---- NNMF iter 1: H1 = xe @ (W^T * rec1r); row sums sx of xe
            z1s = []
            for lo, hi in CHUNKS:
                z = pmm.tile([128, CH], f32, tag="mm")
                nc.tensor.matmul(out=z, lhsT=W2Tpb, rhs=xe[:, lo:hi])
                z1s.append(z)
            # rec2 straight from xe via the host-folded M2
            rec2s = []
            for lo, hi in CHUNKS:
                r = pmm.tile([128, CH], f32, tag="mm")
                nc.tensor.matmul(out=r, lhsT=M2b, rhs=xe[:, lo:hi])
                rec2s.append(r)
            sxs_ps = []
            for lo, hi in CHUNKS:
                sx = pmm.tile([128, CH], f32, tag="mm")
                nc.tensor.matmul(out=sx, lhsT=ones2b, rhs=xe[:, lo:hi])
                sxs_ps.append(sx)
            H1 = hbuf.tile([128, NT], bf16, tag="h")
            for ci, (lo, hi) in enumerate(CHUNKS):
                nc.scalar.activation(out=H1[:, lo:hi], in_=z1s[ci], func=Act.Copy)
            sxs = work.tile([128, NT], f32)
            for ci, (lo, hi) in enumerate(CHUNKS):
                nc.scalar.activation(out=sxs[:, lo:hi], in_=sxs_ps[ci], func=Act.Copy)

            # ---- NNMF iter 2: q2 = xe / rec2; H2 = H1 * (W2T^T q2); s2
            rr2 = work.tile([128, NT], f32, tag="rr2")
            for ci, (lo, hi) in enumerate(CHUNKS):
                nc.vector.reciprocal_approx_fast(out=rr2[:, lo:hi], in_=rec2s[ci])
            q2 = work.tile([128, NT], bf16, tag="q2")
            for ci, (lo, hi) in enumerate(CHUNKS):
                nc.gpsimd.tensor_tensor(
                    out=q2[:, lo:hi], in0=xe[:, lo:hi], in1=rr2[:, lo:hi],
                    op=Alu.mult,
                )
            z2s = []
            z2b = work.tile([128, NT], bf16, tag="z2b")
            for lo, hi in CHUNKS:
                z = pmm.tile([128, CH], f32, tag="mm")
                nc.tensor.matmul(out=z, lhsT=W2Tb, rhs=q2[:, lo:hi])
                z2s.append(z)
            for ci, (lo, hi) in enumerate(CHUNKS):
                nc.scalar.activation(out=z2b[:, lo:hi], in_=z2s[ci], func=Act.Copy)
            H2 = hbuf.tile([128, NT], bf16, tag="h")
            for ci, (lo, hi) in enumerate(CHUNKS):
                nc.vector.tensor_tensor(
                    out=H2[:, lo:hi], in0=H1[:, lo:hi], in1=z2b[:, lo:hi],
                    op=Alu.mult,
                )
            s2_ps = []
            for lo, hi in CHUNKS:
                s2 = pmm.tile([128, CH], f32, tag="mm")
                nc.tensor.matmul(out=s2, lhsT=ones2b, rhs=H2[:, lo:hi])
                s2_ps.append(s2)
            # Bden = sx * s2 (hri denominator); rhoB = 1/Bden
            Bden = work.tile([128, NT], f32)
            for ci, (lo, hi) in enumerate(CHUNKS):
                nc.vector.tensor_tensor(
                    out=Bden[:, lo:hi], in0=sxs[:, lo:hi], in1=s2_ps[ci],
                    op=Alu.mult,
                )
            rhoB = work.tile([128, NT], f32)
            for lo, hi in CHUNKS:
                nc.vector.reciprocal_approx_fast(
                    out=rhoB[:, lo:hi], in_=Bden[:, lo:hi]
                )

            # ---- NNMF iter 3: rec3, q3, H3, s3; hri = (rec3*xe)/B
            rec3s_ps = []
            for lo, hi in CHUNKS:
                r = pmm.tile([128, CH], f32, tag="mm")
                nc.tensor.matmul(out=r, lhsT=W2b, rhs=H2[:, lo:hi])
                rec3s_ps.append(r)
            q3 = work.tile([128, NT], bf16, tag="q3")
            rr3 = work.tile([128, NT], f32, tag="rr3")
            rec3b = work.tile([128, NT], bf16)
            for ci, (lo, hi) in enumerate(CHUNKS):
                nc.vector.reciprocal_approx_fast(out=rr3[:, lo:hi], in_=rec3s_ps[ci])
                nc.scalar.activation(
                    out=rec3b[:, lo:hi], in_=rec3s_ps[ci], func=Act.Copy
                )
            for ci, (lo, hi) in enumerate(CHUNKS):
                nc.gpsimd.tensor_tensor(
                    out=q3[:, lo:hi], in0=xe[:, lo:hi], in1=rr3[:, lo:hi],
                    op=Alu.mult,
                )
            z3s = []
            z3b = work.tile([128, NT], bf16, tag="z3b")
            for lo, hi in CHUNKS:
                z = pmm.tile([128, CH], f32, tag="mm")
                nc.tensor.matmul(out=z, lhsT=W2Tb, rhs=q3[:, lo:hi])
                z3s.append(z)
            for ci, (lo, hi) in enumerate(CHUNKS):
                nc.scalar.activation(out=z3b[:, lo:hi], in_=z3s[ci], func=Act.Copy)
            H3 = hbuf.tile([128, NT], bf16, tag="h")
            for ci, (lo, hi) in enumerate(CHUNKS):
                nc.vector.tensor_tensor(
                    out=H3[:, lo:hi], in0=H2[:, lo:hi], in1=z3b[:, lo:hi],
                    op=Alu.mult,
                )
            # hri_a = rec3 * xe on DVE (all-bf16: probes the 2x fast mode).
            # owa/owc DMAs are anchored on H2 via tiny writes into the
            # destination tiles so the scheduler cannot hoist them into the
            # input-DMA window (they'd steal bandwidth from xT/ewT).
            hria = work.tile([128, NT], bf16)
            owa = const.tile([128, FIN], bf16)
            owc = const.tile([64, FIN], bf16)
            nc.gpsimd.tensor_copy(out=owa[:, 0:1], in_=H2[:, 0:1])
            nc.gpsimd.tensor_copy(out=owc[:, 0:1], in_=H2[0:64, 0:1])
            nc.gpsimd.dma_start(out=owa, in_=d_owa[:, :])
            nc.gpsimd.dma_start(out=owc, in_=d_owc[:, :])
            # hria = rec3 * xe (all-bf16 2x TT); hri = hria * rhoB
            for lo, hi in CHUNKS:
                nc.vector.tensor_tensor(
                    out=hria[:, lo:hi], in0=rec3b[:, lo:hi], in1=xe[:, lo:hi],
                    op=Alu.mult,
                )
            hri = work.tile([128, NT], bf16)
            for lo, hi in CHUNKS:
                nc.gpsimd.tensor_tensor(
                    out=hri[:, lo:hi], in0=hria[:, lo:hi], in1=rhoB[:, lo:hi],
                    op=Alu.mult,
                )
            s3_ps = []
            for lo, hi in CHUNKS:
                s3 = pmm.tile([128, CH], f32, tag="mm")
                nc.tensor.matmul(out=s3, lhsT=ones2b, rhs=H3[:, lo:hi])
                s3_ps.append(s3)
            u0 = work.tile([128, NT], f32)
            for ci, (lo, hi) in enumerate(CHUNKS):
                nc.vector.reciprocal_approx_fast(out=u0[:, lo:hi], in_=s3_ps[ci])

            # ---- alpha fixed point: 4 accumulation passes, 3 v/g rounds
            vv = pt.tile([128, 4], f32, tag="v", bufs=1)
            c_p = work.tile([128, 1], f32)
            c_cc = work.tile([128, 1], f32)
            t_prev = None
            g_ps = None
            for it in range(4):
                lastit = it == 3
                t = tbuf.tile([128, NT], bf16, tag="t")
                in0 = H3 if it == 0 else t_prev
                macc = []
                mbs = []
                vps = vv[:, 0:1]
                vcs = vv[:, 1:2]
                for ci, (lo, hi) in enumerate(CHUNKS):
                    in1 = u0[:, lo:hi] if it == 0 else g_ps[ci]
                    m = work.tile([128, 1], f32, tag=f"m{it}{ci}")
                    nc.vector.scalar_tensor_tensor(
                        out=t[:, lo:hi], in0=in0[:, lo:hi], scalar=1.0,
                        in1=in1, op0=Alu.mult, op1=Alu.mult, accum_out=m,
                    )
                    macc.append(m)
                    if lastit:
                        continue
                    # cast (on ACT, off the DVE serial path) + accumulated v
                    # matmul immediately per chunk so the v chain overlaps
                    # the remaining STTs
                    mb = work.tile([128, 1], bf16, tag=f"mb{it}{ci}")
                    nc.scalar.activation(out=mb, in_=m, func=Act.Copy)
                    mbs.append(mb)
                    if ci < 2:
                        nc.tensor.matmul(
                            out=vps, lhsT=W2b, rhs=mb,
                            start=(ci == 0), stop=(ci == 1),
                            skip_group_check=True,
                        )
                    else:
                        nc.tensor.matmul(
                            out=vcs, lhsT=Wstk2b, rhs=mb, skip_group_check=True
                        )
                t_prev = t
                if lastit:
                    nc.vector.tensor_tensor(
                        out=c_p, in0=macc[0], in1=macc[1], op=Alu.add
                    )
                    nc.vector.tensor_copy(out=c_cc, in_=macc[2])
                    break
                v_p = work.tile([128, 1], f32, tag="v_p")
                v_c = work.tile([128, 1], f32, tag="v_c")
                nc.vector.reciprocal_approx_fast(out=v_p, in_=vps)
                nc.vector.reciprocal_approx_fast(out=v_c, in_=vcs)
                vblk = work.tile([128, 128], bf16, tag="vblk")
                vblkC = work.tile([128, 128], bf16, tag="vblkC")
                nc.vector.tensor_scalar(
                    out=vblk, in0=ones2b, scalar1=v_p, scalar2=None, op0=Alu.mult
                )
                nc.scalar.activation(
                    out=vblkC, in_=ones2b, func=Act.Copy, scale=v_c
                )
                g_ps = []
                for ci, (lo, hi) in enumerate(CHUNKS):
                    g = pmm.tile([128, CH], f32, tag="mm")
                    nc.tensor.matmul(
                        out=g, lhsT=(vblkC if ci == 2 else vblk),
                        rhs=hri[:, lo:hi],
                    )
                    g_ps.append(g)

            # fold the C accumulator's split halves: c_c[f] = acc[f]+acc[64+f]
            c_ccb = work.tile([128, 1], bf16)
            nc.vector.tensor_copy(out=c_ccb, in_=c_cc)
            fc = vv[0:64, 2:3]
            nc.tensor.matmul(out=fc, lhsT=idstkb, rhs=c_ccb, skip_group_check=True)
            c_c = work.tile([64, 1], bf16)
            nc.scalar.activation(out=c_c, in_=fc, func=Act.Copy)

            # ---- output projection partial: y_row = c^T @ owT  [1, FIN]
            c_pr = work.tile([128, 1], bf16)
            nc.vector.tensor_copy(out=c_pr, in_=c_p)
            py0 = pmm.tile([1, 512], f32, tag="mm")
            py1 = pmm.tile([1, 256], f32, tag="mm")
            for py, (lo, hi) in ((py0, (0, 512)), (py1, (512, FIN))):
                nc.tensor.matmul(
                    out=py, lhsT=c_pr, rhs=owa[:, lo:hi], start=True, stop=False
                )
                nc.tensor.matmul(
                    out=py, lhsT=c_c, rhs=owc[:, lo:hi], start=False, stop=True
                )
            y_sb = work.tile([1, FIN], f32)
            nc.scalar.activation(out=y_sb[:, 0:512], in_=py0, func=Act.Copy)
            nc.scalar.activation(out=y_sb[:, 512:FIN], in_=py1, func=Act.Copy)
            nc.sync.dma_start(out=d_y[:, :], in_=y_sb)

    nc.finalize()
    return nc


def _make_in_maps(x, embed_w, embed_b, nnmf_w, out_w):
    import ml_dtypes

    def to_fp8(a):
        return np.ascontiguousarray(a).astype(ml_dtypes.float8_e4m3fn)

    def to_bf16(a):
        return np.ascontiguousarray(a).astype(ml_dtypes.bfloat16)

    # host W-prep (shared across heads/cores)
    Wn = nnmf_w / np.maximum(nnmf_w.sum(1, keepdims=True), EPS)  # [f,d]
    rec1r = 64.0 / np.maximum(Wn.sum(0), EPS)                    # [d]
    W2Tp = Wn.T * rec1r[:, None]                                 # [d,f]
    M2 = W2Tp @ Wn                                               # [d,d']
    W2T = Wn.T                                                   # [d,f]

    def blkdiag(A):
        Z = np.zeros((128, 128), np.float32)
        Z[0:64, 0:64] = A
        Z[64:128, 64:128] = A
        return Z

    idstk = np.zeros((128, 64), np.float32)
    for k in range(128):
        idstk[k, k % 64] = 1.0
    ones2 = blkdiag(np.ones((64, 64), np.float32))
    Wstk2 = np.zeros((128, 128), np.float32)
    Wstk2[0:64, 0:64] = Wn
    Wstk2[64:128, 0:64] = Wn
    Wstk2[0:64, 64:128] = Wn
    Wstk2[64:128, 64:128] = Wn

    wbpack = np.concatenate(
        [blkdiag(W2Tp), blkdiag(M2), blkdiag(Wn), blkdiag(W2T),
         Wstk2, idstk, ones2],
        axis=1,
    ).astype(np.float32)
    wb = to_bf16(wbpack)

    in_maps = []
    for c in range(NCORES):
        b = c // 4
        hg = c % 4
        esl = slice(EPC * hg, EPC * (hg + 1))
        # xT packed [128, KT, S]: (p, k, t) = x[b, t, 128k+p]
        xT = np.ascontiguousarray(
            x[b].T.reshape(KT, 128, S).transpose(1, 0, 2)
        )
        ewT = np.ascontiguousarray(
            embed_w[esl, :].T.reshape(KT, 128, EPC).transpose(1, 0, 2)
        )
        ewc2 = np.zeros((128, KT, 128), np.float32)
        ewc2[:, :, 64:128] = ewT[:, :, 128:192]
        ebm = embed_b[esl] - MIN_POS
        wf = np.zeros((128, 2), np.float32)
        wf[:, 0] = ebm[0:128]
        wf[0:64, 1] = ebm[128:192]
        wf[64:128, 1] = ebm[128:192]
        owT = out_w[:, esl].T  # [192, FIN]
        in_maps.append({
            "xT": to_fp8(xT),
            "ewT": to_fp8(ewT * 64.0),
            "ewc2": to_fp8(ewc2 * 64.0),
            "wb": wb,
            "wf": wf,
            "owa": to_bf16(owT[0:128, :]),
            "owc": to_bf16(owT[128:192, :]),
        })
    return in_maps


def _ensure_ntff_hook():
    """The agent image's antenv lacks axon_hooks; synthesize it so
    run_bass_kernel_spmd(trace=True) can reach the ctypes NTFF hook."""
    import sys as _sys
    import types

    if "antenv.axon_hooks" in _sys.modules:
        return
    mod = types.ModuleType("antenv.axon_hooks")
    holder = [None]
    mod.set_axon_ntff_profile_hook = lambda h: holder.__setitem__(0, h)
    mod.get_axon_ntff_profile_hook = lambda: holder[0]
    _sys.modules["antenv.axon_hooks"] = mod
    try:
        import antenv

        antenv.axon_hooks = mod
    except ImportError:
        pass
    from trn_agent_boot.trn_boot import _ntff_profile_via_ctypes

    mod.set_axon_ntff_profile_hook(
        _ntff_profile_via_ctypes("/opt/axon/libaxon_pjrt.so")
    )


def _run(inputs, trace=False):
    from concourse import bass_utils

    if trace:
        _ensure_ntff_hook()
    if "nc" not in _CACHE:
        _CACHE["nc"] = _build_nc()
    nc = _CACHE["nc"]
    in_maps = _make_in_maps(
        inputs["x"].astype(np.float32),
        inputs["embed_w"].astype(np.float32),
        inputs["embed_b"].astype(np.float32),
        inputs["nnmf_w"].astype(np.float32),
        inputs["out_w"].astype(np.float32),
    )
    res = bass_utils.run_bass_kernel_spmd(
        nc, in_maps, core_ids=list(range(NCORES)), trace=trace
    )
    out_b = inputs["out_b"].astype(np.float32)
    y = np.zeros((B, S, FIN), np.float32)
    for bi in range(B):
        acc = np.zeros((FIN,), np.float64)
        for c in range(4 * bi, 4 * bi + 4):
            arr = np.asarray(res.results[c]["y"])  # [1, FIN]
            acc += arr.reshape(FIN)
        y[bi, :, :] = (acc + out_b).astype(np.float32)[None, :]
    return y, res


def kernel(**inputs):
    y, _ = _run(inputs, trace=False)
    return y


# revision 47
# speedup vs baseline: 1.0204x; 1.0143x over previous
"""AlphaMixerAttentionHeads TRN2 kernel, v2.

Algebraic structure (inherited from the validated baseline):
 - alpha is i-independent, so it collapses to a per-(b,h) vector u over o;
   the output is constant across sequence positions and equals
   m_3 = sum_o H3[:,o] u_3[o].
 - W rows are L1-normalized, so all per-token scales cancel through the
   NNMF recurrence, which runs on raw clipped xe.

v2 changes vs the baseline:
 - All W preparation (row-normalize, transpose, iter-1 fold, M2 = W2Tp@W2)
   happens on the HOST in numpy; the device receives ready bf16 lhsT tiles.
   This removes the on-device transpose/normalize chain that sat on the
   critical path between the wpk DMA and the first NNMF matmul.
 - rec2 = M2^T xe is computed straight from xe (M2 folded on host), so
   iteration 2 does not wait for the H1 copy.
 - DMA queues: x + ewc2 on the sync queue, ewT + const packs on the scalar
   queue (smallest/earliest-needed first), out-projection weights on the
   gpsimd queue triggered mid-kernel -- the embed matmuls no longer wait on
   the (large, late-needed) out-projection DMAs.
 - q = xe / rec and hri = hri_a / B are single DVE divide ops (no separate
   reciprocal+multiply).
 - alpha rounds: per-chunk accumulator matmuls (W^T m accumulated in PSUM
   across chunks), vblk built on DVE, g for chunks 1-2 copied to bf16 so the
   scalar_tensor_tensor accumulation runs in the DVE 2-byte fast mode; the
   t tiles are bf16.

Sharding: 8 cores; core c handles batch c//4 and heads 3*(c%4)..3*(c%4)+2.
Host sums 4 partial output projections per batch, adds out_b, broadcasts
over the sequence axis.

On-core layout: [feature, token], one [128, 1536] tile set: cols 0..1023 =
heads A,B (partitions 0-63 = A, 64-127 = B); cols 1024..1535 = head C
split-token (partitions 0-63 = tokens 0-511, 64-127 = tokens 512-1023).
"""

import sys

sys.path.insert(0, "/opt/trn_rl_repo")

import numpy as np

B, S, FIN, E, H = 2, 1024, 768, 768, 12
DH = 64
HPC = 3          # heads per core
EPC = HPC * DH   # embed channels per core (192)
NCORES = 8
MIN_POS = 1e-6
EPS = 1e-20
NT = 1536        # merged token columns: 1024 pair + 512 C-split
KT = FIN // 128  # 6 contraction tiles for the embed matmul
CH = 512         # pipeline chunk columns
# wb packed bf16 columns: W2Tpb | M2b | W2b | W2Tb | Wstk2b | idstkb | ones2b
WB_COLS = 128 + 128 + 128 + 128 + 128 + 64 + 128

_CACHE = {}


def _build_nc():
    import concourse.bacc as bacc
    import concourse.mybir as mybir
    from concourse.tile import TileContext

    f32 = mybir.dt.float32
    f32r = mybir.dt.float32r
    bf16 = mybir.dt.bfloat16
    Alu = mybir.AluOpType
    Act = mybir.ActivationFunctionType

    nc = bacc.Bacc()

    fp8 = mybir.dt.float8e4
    d_xT = nc.declare_dram_parameter("xT", [128, KT, S], fp8, isOutput=False)
    d_ewT = nc.declare_dram_parameter("ewT", [128, KT, EPC], fp8, isOutput=False)
    d_ewc2 = nc.declare_dram_parameter("ewc2", [128, KT, 128], fp8, isOutput=False)
    d_wb = nc.declare_dram_parameter("wb", [128, WB_COLS], bf16, isOutput=False)
    d_wf = nc.declare_dram_parameter("wf", [128, 2], f32, isOutput=False)
    d_owa = nc.declare_dram_parameter("owa", [128, FIN], bf16, isOutput=False)
    d_owc = nc.declare_dram_parameter("owc", [64, FIN], bf16, isOutput=False)
    d_y = nc.declare_dram_parameter("y", [1, FIN], f32, isOutput=True)

    CHUNKS = ((0, 512), (512, 1024), (1024, 1536))

    with TileContext(nc) as tc:
        with (
            tc.tile_pool(name="const", bufs=1) as const,
            tc.tile_pool(name="xch", bufs=3) as xch,
            tc.tile_pool(name="work", bufs=1) as work,
            tc.tile_pool(name="hbuf", bufs=2) as hbuf,
            tc.tile_pool(name="tbuf", bufs=2) as tbuf,
            tc.tile_pool(name="pmm", bufs=4, space="PSUM") as pmm,
            tc.tile_pool(name="pt", bufs=1, space="PSUM") as pt,
        ):
            # ---- DMA triggers. sync queue: x tiles; scalar queue: embed
            # weights + const packs. owa/owc go on the gpsimd queue later.
            xts = []
            for i in range(3):
                xt = xch.tile([128, 2, S], fp8, tag="xch")
                nc.sync.dma_start(out=xt, in_=d_xT[:, 2 * i:2 * i + 2, :])
                xts.append(xt)
            ewT_sb = const.tile([128, KT, EPC], fp8)
            nc.scalar.dma_start(out=ewT_sb, in_=d_ewT[:, :, :])
            ewc2_sb = const.tile([128, KT, 128], fp8)
            nc.scalar.dma_start(out=ewc2_sb, in_=d_ewc2[:, :, :])
            wb = const.tile([128, WB_COLS], bf16)
            nc.scalar.dma_start(out=wb, in_=d_wb[:, :])
            wf = const.tile([128, 2], f32)
            nc.scalar.dma_start(out=wf, in_=d_wf[:, :])

            W2Tpb = wb[:, 0:128]
            M2b = wb[:, 128:256]
            W2b = wb[:, 256:384]
            W2Tb = wb[:, 384:512]
            Wstk2b = wb[:, 512:640]
            idstkb = wb[:, 640:704]
            ones2b = wb[:, 704:832]
            ebm_p = wf[:, 0:1]
            ebm_c2 = wf[:, 1:2]

            # ---- embed matmuls: ep = pair heads [128, 1024];
            # psC = head C split-token [128, 512].
            ep = pmm.tile([128, 1024], f32, tag="ep", bufs=1)
            psC = pmm.tile([128, CH], f32, tag="pc", bufs=1)
            DR = mybir.MatmulPerfMode.DoubleRow
            for g in range(3):
                xt = xts[g]
                lhsP2 = ewT_sb[:, 2 * g:2 * g + 2, 0:128]
                st = dict(start=(g == 0), stop=(g == 2), perf_mode=DR)
                nc.tensor.matmul(
                    out=ep[:, 0:512], lhsT=lhsP2, rhs=xt[:, :, 0:512], **st
                )
                nc.tensor.matmul(
                    out=ep[:, 512:1024], lhsT=lhsP2, rhs=xt[:, :, 512:1024], **st
                )
                nc.tensor.matmul(
                    out=psC, lhsT=ewc2_sb[:, 2 * g:2 * g + 2, :],
                    rhs=xt[:, :, 512:1024], skip_group_check=True, **st,
                )
                nc.tensor.matmul(
                    out=psC[0:64, :], lhsT=ewT_sb[:, 2 * g:2 * g + 2, 128:192],
                    rhs=xt[:, :, 0:512], start=False, stop=(g == 2),
                    perf_mode=DR, skip_group_check=True,
                )

            # ---- xe = relu(embed + eb - MIN_POS) on ACT (bias pre-folded)
            xe = work.tile([128, NT], bf16)
            nc.scalar.activation(
                out=xe[:, 0:512], in_=ep[:, 0:512], func=Act.Relu,
                bias=ebm_p, scale=1.0 / 64.0,
            )
            nc.scalar.activation(
                out=xe[:, 512:1024], in_=ep[:, 512:1024], func=Act.Relu,
                bias=ebm_p, scale=1.0 / 64.0,
            )
            nc.scalar.activation(
                out=xe[:, 1024:1536], in_=psC, func=Act.Relu,
                bias=ebm_c2, scale=1.0 / 64.0,
            )

            # ---- NNMF iter 1: H1 = xe @ (W^T * rec1r); row sums sx of xe
            z1s = []
            for lo, hi in CHUNKS:
                z = pmm.tile([128, CH], f32, tag="mm")
                nc.tensor.matmul(out=z, lhsT=W2Tpb, rhs=xe[:, lo:hi])
                z1s.append(z)
            # rec2 straight from xe via the host-folded M2
            rec2s = []
            for lo, hi in CHUNKS:
                r = pmm.tile([128, CH], f32, tag="mm")
                nc.tensor.matmul(out=r, lhsT=M2b, rhs=xe[:, lo:hi])
                rec2s.append(r)
            sxs_ps = []
            for lo, hi in CHUNKS:
                sx = pmm.tile([128, CH], f32, tag="mm")
                nc.tensor.matmul(out=sx, lhsT=ones2b, rhs=xe[:, lo:hi])
                sxs_ps.append(sx)
            H1 = hbuf.tile([128, NT], bf16, tag="h")
            for ci, (lo, hi) in enumerate(CHUNKS):
                nc.scalar.activation(out=H1[:, lo:hi], in_=z1s[ci], func=Act.Copy)
            sxs = work.tile([128, NT], f32)
            for ci, (lo, hi) in enumerate(CHUNKS):
                nc.scalar.activation(out=sxs[:, lo:hi], in_=sxs_ps[ci], func=Act.Copy)

            # ---- NNMF iter 2: q2 = xe / rec2; H2 = H1 * (W2T^T q2); s2
            rr2 = work.tile([128, NT], f32, tag="rr2")
            for ci, (lo, hi) in enumerate(CHUNKS):
                nc.vector.reciprocal_approx_fast(out=rr2[:, lo:hi], in_=rec2s[ci])
            q2 = work.tile([128, NT], bf16, tag="q2")
            for ci, (lo, hi) in enumerate(CHUNKS):
                nc.gpsimd.tensor_tensor(
                    out=q2[:, lo:hi], in0=xe[:, lo:hi], in1=rr2[:, lo:hi],
                    op=Alu.mult,
                )
            z2s = []
            z2b = work.tile([128, NT], bf16, tag="z2b")
            for lo, hi in CHUNKS:
                z = pmm.tile([128, CH], f32, tag="mm")
                nc.tensor.matmul(out=z, lhsT=W2Tb, rhs=q2[:, lo:hi])
                z2s.append(z)
            for ci, (lo, hi) in enumerate(CHUNKS):
                nc.scalar.activation(out=z2b[:, lo:hi], in_=z2s[ci], func=Act.Copy)
            H2 = hbuf.tile([128, NT], bf16, tag="h")
            for ci, (lo, hi) in enumerate(CHUNKS):
                nc.vector.tensor_tensor(
                    out=H2[:, lo:hi], in0=H1[:, lo:hi], in1=z2b[:, lo:hi],
                    op=Alu.mult,
                )
            s2_ps = []
            for lo, hi in CHUNKS:
                s2 = pmm.tile([128, CH], f32, tag="mm")
                nc.tensor.matmul(out=s2, lhsT=ones2b, rhs=H2[:, lo:hi])
                s2_ps.append(s2)
            # Bden = sx * s2 (hri denominator); rhoB = 1/Bden
            Bden = work.tile([128, NT], f32)
            for ci, (lo, hi) in enumerate(CHUNKS):
                nc.vector.tensor_tensor(
                    out=Bden[:, lo:hi], in0=sxs[:, lo:hi], in1=s2_ps[ci],
                    op=Alu.mult,
                )
            rhoB = work.tile([128, NT], f32)
            for lo, hi in CHUNKS:
                nc.vector.reciprocal_approx_fast(
                    out=rhoB[:, lo:hi], in_=Bden[:, lo:hi]
                )

            # ---- NNMF iter 3: rec3, q3, H3, s3; hri = (rec3*xe)/B
            rec3s_ps = []
            for lo, hi in CHUNKS:
                r = pmm.tile([128, CH], f32, tag="mm")
                nc.tensor.matmul(out=r, lhsT=W2b, rhs=H2[:, lo:hi])
                rec3s_ps.append(r)
            q3 = work.tile([128, NT], bf16, tag="q3")
            rr3 = work.tile([128, NT], f32, tag="rr3")
            rec3b = work.tile([128, NT], bf16)
            for ci, (lo, hi) in enumerate(CHUNKS):
                nc.vector.reciprocal_approx_fast(out=rr3[:, lo:hi], in_=rec3s_ps[ci])
                nc.scalar.activation(
                    out=rec3b[:, lo:hi], in_=rec3s_ps[ci], func=Act.Copy
                )
            for ci, (lo, hi) in enumerate(CHUNKS):
                nc.gpsimd.tensor_tensor(
                    out=q3[:, lo:hi], in0=xe[:, lo:hi], in1=rr3[:, lo:hi],
                    op=Alu.mult,
                )
            z3s = []
            z3b = work.tile([128, NT], bf16, tag="z3b")
            for lo, hi in CHUNKS:
                z = pmm.tile([128, CH], f32, tag="mm")
                nc.tensor.matmul(out=z, lhsT=W2Tb, rhs=q3[:, lo:hi])
                z3s.append(z)
            for ci, (lo, hi) in enumerate(CHUNKS):
                nc.scalar.activation(out=z3b[:, lo:hi], in_=z3s[ci], func=Act.Copy)
            H3 = hbuf.tile([128, NT], bf16, tag="h")
            for ci, (lo, hi) in enumerate(CHUNKS):
                nc.vector.tensor_tensor(
                    out=H3[:, lo:hi], in0=H2[:, lo:hi], in1=z3b[:, lo:hi],
                    op=Alu.mult,
                )
            # hri_a = rec3 * xe on DVE (all-bf16: probes the 2x fast mode).
            # owa/owc DMAs are anchored on H2 via tiny writes into the
            # destination tiles so the scheduler cannot hoist them into the
            # input-DMA window (they'd steal bandwidth from xT/ewT).
            hria = work.tile([128, NT], bf16)
            owa = const.tile([128, FIN], bf16)
            owc = const.tile([64, FIN], bf16)
            nc.gpsimd.tensor_copy(out=owa[:, 0:1], in_=H2[:, 0:1])
            nc.gpsimd.tensor_copy(out=owc[:, 0:1], in_=H2[0:64, 0:1])
            nc.gpsimd.dma_start(out=owa, in_=d_owa[:, :])
            nc.gpsimd.dma_start(out=owc, in_=d_owc[:, :])
            # hria = rec3 * xe (all-bf16 2x TT); hri = hria * rhoB
            for lo, hi in CHUNKS:
                nc.vector.tensor_tensor(
                    out=hria[:, lo:hi], in0=rec3b[:, lo:hi], in1=xe[:, lo:hi],
                    op=Alu.mult,
                )
            hri = work.tile([128, NT], bf16)
            for lo, hi in CHUNKS:
                nc.gpsimd.tensor_tensor(
                    out=hri[:, lo:hi], in0=hria[:, lo:hi], in1=rhoB[:, lo:hi],
                    op=Alu.mult,
                )
            s3_ps = []
            for lo, hi in CHUNKS:
                s3 = pmm.tile([128, CH], f32, tag="mm")
                nc.tensor.matmul(out=s3, lhsT=ones2b, rhs=H3[:, lo:hi])
                s3_ps.append(s3)
            u0 = work.tile([128, NT], f32)
            for ci, (lo, hi) in enumerate(CHUNKS):
                nc.vector.reciprocal_approx_fast(out=u0[:, lo:hi], in_=s3_ps[ci])

            # ---- alpha fixed point: 4 accumulation passes, 3 v/g rounds
            vv = pt.tile([128, 4], f32, tag="v", bufs=1)
            t_prev = None
            g_ps = None
            for it in range(4):
                lastit = it == 3
                t = tbuf.tile([128, NT], bf16, tag="t")
                in0 = H3 if it == 0 else t_prev
                macc = {}
                vps = vv[:, 0:1]
                vcs = vv[:, 1:2]
                # final pass runs the C chunk first so the C fold chain
                # (cast -> fold matmul -> c_c) hides under the pair STTs
                order = (2, 0, 1) if lastit else (0, 1, 2)
                for ci in order:
                    lo, hi = CHUNKS[ci]
                    in1 = u0[:, lo:hi] if it == 0 else g_ps[ci]
                    m = work.tile([128, 1], f32, tag=f"m{it}{ci}")
                    nc.vector.scalar_tensor_tensor(
                        out=t[:, lo:hi], in0=in0[:, lo:hi], scalar=1.0,
                        in1=in1, op0=Alu.mult, op1=Alu.mult, accum_out=m,
                    )
                    macc[ci] = m
                    if lastit:
                        if ci == 2:
                            # fold C's split halves immediately:
                            # c_c[f] = acc[f] + acc[64+f]
                            c_ccb = work.tile([128, 1], bf16)
                            nc.vector.tensor_copy(out=c_ccb, in_=m)
                            fc = vv[0:64, 2:3]
                            nc.tensor.matmul(
                                out=fc, lhsT=idstkb, rhs=c_ccb,
                                skip_group_check=True,
                            )
                            c_c = work.tile([64, 1], bf16)
                            nc.scalar.activation(out=c_c, in_=fc, func=Act.Copy)
                        continue
                    # cast (on ACT, off the DVE serial path) + accumulated v
                    # matmul immediately per chunk so the v chain overlaps
                    # the remaining STTs
                    mb = work.tile([128, 1], bf16, tag=f"mb{it}{ci}")
                    nc.scalar.activation(out=mb, in_=m, func=Act.Copy)
                    if ci < 2:
                        nc.tensor.matmul(
                            out=vps, lhsT=W2b, rhs=mb,
                            start=(ci == 0), stop=(ci == 1),
                            skip_group_check=True,
                        )
                    else:
                        nc.tensor.matmul(
                            out=vcs, lhsT=Wstk2b, rhs=mb, skip_group_check=True
                        )
                t_prev = t
                if lastit:
                    # pair partial in bf16 directly (single add+cast op)
                    c_pr = work.tile([128, 1], bf16)
                    nc.vector.tensor_tensor(
                        out=c_pr, in0=macc[0], in1=macc[1], op=Alu.add
                    )
                    break
                v_p = work.tile([128, 1], f32, tag="v_p")
                v_c = work.tile([128, 1], f32, tag="v_c")
                nc.vector.reciprocal_approx_fast(out=v_p, in_=vps)
                nc.vector.reciprocal_approx_fast(out=v_c, in_=vcs)
                vblk = work.tile([128, 128], bf16, tag="vblk")
                vblkC = work.tile([128, 128], bf16, tag="vblkC")
                nc.vector.tensor_scalar(
                    out=vblk, in0=ones2b, scalar1=v_p, scalar2=None, op0=Alu.mult
                )
                nc.scalar.activation(
                    out=vblkC, in_=ones2b, func=Act.Copy, scale=v_c
                )
                g_ps = []
                for ci, (lo, hi) in enumerate(CHUNKS):
                    g = pmm.tile([128, CH], f32, tag="mm")
                    nc.tensor.matmul(
                        out=g, lhsT=(vblkC if ci == 2 else vblk),
                        rhs=hri[:, lo:hi],
                    )
                    g_ps.append(g)

            # ---- output projection partial: y_row = c^T @ owT  [1, FIN]
            py0 = pmm.tile([1, 512], f32, tag="mm")
            py1 = pmm.tile([1, 256], f32, tag="mm")
            for py, (lo, hi) in ((py0, (0, 512)), (py1, (512, FIN))):
                nc.tensor.matmul(
                    out=py, lhsT=c_pr, rhs=owa[:, lo:hi], start=True, stop=False
                )
                nc.tensor.matmul(
                    out=py, lhsT=c_c, rhs=owc[:, lo:hi], start=False, stop=True
                )
            y_sb = work.tile([1, FIN], f32)
            nc.scalar.activation(out=y_sb[:, 0:512], in_=py0, func=Act.Copy)
            nc.scalar.activation(out=y_sb[:, 512:FIN], in_=py1, func=Act.Copy)
            nc.sync.dma_start(out=d_y[:, :], in_=y_sb)

    nc.finalize()
    return nc


def _make_in_maps(x, embed_w, embed_b, nnmf_w, out_w):
    import ml_dtypes

    def to_fp8(a):
        return np.ascontiguousarray(a).astype(ml_dtypes.float8_e4m3fn)

    def to_bf16(a):
        return np.ascontiguousarray(a).astype(ml_dtypes.bfloat16)

    # host W-prep (shared across heads/cores)
    Wn = nnmf_w / np.maximum(nnmf_w.sum(1, keepdims=True), EPS)  # [f,d]
    rec1r = 64.0 / np.maximum(Wn.sum(0), EPS)                    # [d]
    W2Tp = Wn.T * rec1r[:, None]                                 # [d,f]
    M2 = W2Tp @ Wn                                               # [d,d']
    W2T = Wn.T                                                   # [d,f]

    def blkdiag(A):
        Z = np.zeros((128, 128), np.float32)
        Z[0:64, 0:64] = A
        Z[64:128, 64:128] = A
        return Z

    idstk = np.zeros((128, 64), np.float32)
    for k in range(128):
        idstk[k, k % 64] = 1.0
    ones2 = blkdiag(np.ones((64, 64), np.float32))
    Wstk2 = np.zeros((128, 128), np.float32)
    Wstk2[0:64, 0:64] = Wn
    Wstk2[64:128, 0:64] = Wn
    Wstk2[0:64, 64:128] = Wn
    Wstk2[64:128, 64:128] = Wn

    wbpack = np.concatenate(
        [blkdiag(W2Tp), blkdiag(M2), blkdiag(Wn), blkdiag(W2T),
         Wstk2, idstk, ones2],
        axis=1,
    ).astype(np.float32)
    wb = to_bf16(wbpack)

    in_maps = []
    for c in range(NCORES):
        b = c // 4
        hg = c % 4
        esl = slice(EPC * hg, EPC * (hg + 1))
        # xT packed [128, KT, S]: (p, k, t) = x[b, t, 128k+p]
        xT = np.ascontiguousarray(
            x[b].T.reshape(KT, 128, S).transpose(1, 0, 2)
        )
        ewT = np.ascontiguousarray(
            embed_w[esl, :].T.reshape(KT, 128, EPC).transpose(1, 0, 2)
        )
        ewc2 = np.zeros((128, KT, 128), np.float32)
        ewc2[:, :, 64:128] = ewT[:, :, 128:192]
        ebm = embed_b[esl] - MIN_POS
        wf = np.zeros((128, 2), np.float32)
        wf[:, 0] = ebm[0:128]
        wf[0:64, 1] = ebm[128:192]
        wf[64:128, 1] = ebm[128:192]
        owT = out_w[:, esl].T  # [192, FIN]
        in_maps.append({
            "xT": to_fp8(xT),
            "ewT": to_fp8(ewT * 64.0),
            "ewc2": to_fp8(ewc2 * 64.0),
            "wb": wb,
            "wf": wf,
            "owa": to_bf16(owT[0:128, :]),
            "owc": to_bf16(owT[128:192, :]),
        })
    return in_maps


def _ensure_ntff_hook():
    """The agent image's antenv lacks axon_hooks; synthesize it so
    run_bass_kernel_spmd(trace=True) can reach the ctypes NTFF hook."""
    import sys as _sys
    import types

    if "antenv.axon_hooks" in _sys.modules:
        return
    mod = types.ModuleType("antenv.axon_hooks")
    holder = [None]
    mod.set_axon_ntff_profile_hook = lambda h: holder.__setitem__(0, h)
    mod.get_axon_ntff_profile_hook = lambda: holder[0]
    _sys.modules["antenv.axon_hooks"] = mod
    try:
        import antenv

        antenv.axon_hooks = mod
    except ImportError:
        pass
    from trn_agent_boot.trn_boot import _ntff_profile_via_ctypes

    mod.set_axon_ntff_profile_hook(
        _ntff_profile_via_ctypes("/opt/axon/libaxon_pjrt.so")
    )


def _run(inputs, trace=False):
    from concourse import bass_utils

    if trace:
        _ensure_ntff_hook()
    if "nc" not in _CACHE:
        _CACHE["nc"] = _build_nc()
    nc = _CACHE["nc"]
    in_maps = _make_in_maps(
        inputs["x"].astype(np.float32),
        inputs["embed_w"].astype(np.float32),
        inputs["embed_b"].astype(np.float32),
        inputs["nnmf_w"].astype(np.float32),
        inputs["out_w"].astype(np.float32),
    )
    res = bass_utils.run_bass_kernel_spmd(
        nc, in_maps, core_ids=list(range(NCORES)), trace=trace
    )
    out_b = inputs["out_b"].astype(np.float32)
    y = np.zeros((B, S, FIN), np.float32)
    for bi in range(B):
        acc = np.zeros((FIN,), np.float64)
        for c in range(4 * bi, 4 * bi + 4):
            arr = np.asarray(res.results[c]["y"])  # [1, FIN]
            acc += arr.reshape(FIN)
        y[bi, :, :] = (acc + out_b).astype(np.float32)[None, :]
    return y, res


def kernel(**inputs):
    y, _ = _run(inputs, trace=False)
    return y
